# revision 2
# baseline (speedup 1.0000x reference)
# kernel_new.py — DecoderSourceTarget via SWDGE dst-gather + PE one-hot src
# expansion on 8 Trainium2 NeuronCores.
#
# reference:
#   src = x[eli[0], :128]; dst = x[eli[1], 128:]
#   out = sigmoid(sum(src * dst, -1))[:, None]        # [E, 1] f32
#
# Design:
#   - Edges are grouped into cells (src_tile[128 rows] x dst_chunk[25000
#     rows]).  Per dst_chunk, cells are sorted by size and dealt round-robin
#     to the 8 cores, so position j on every core has the same capacity
#     cap[b][j] (max of the 8 rank-adjacent cells, ~2% padding).  The
#     program is identical across cores; only tensor contents differ.
#   - dst side: per dst-chunk region, slot-contiguous SWDGE dma_gather
#     (256B rows, 4 queues, reset every 4 slabs) — the DMA-transfer cost of
#     these random 256B packets (~25.7ns engine time each) is the roofline.
#   - src side: gathered by the PE.  Each core keeps its ~394 position
#     tiles (128 rows x 128 feats bf16) resident in SBUF; a one-hot matrix
#     (DVE is_equal of srcloc vs an iota column) selects rows:
#     psum[slots,128f] = onehot[128rows, slots].T @ xs_tile[128rows, 128f].
#   - DVE multiplies psum (src) x gathered dst tile and reduces the feature
#     axis per 1024-slot stage; sigmoid once at the end.
#   - Host maps slots back to edge order (numpy bookkeeping).

import numpy as np

N_NODES = 100000
HALF = 128
N_EDGES = 1000000
N_CORES = 8
TILE = 128
N_TILES = (N_NODES + TILE - 1) // TILE       # 782
N_BETA = 4
BCHUNK = N_NODES // N_BETA                   # 25000
STAGE = 1024                                 # slots per psum stage (2 banks)
SLAB = 3072                                  # slots per dst gather
ONEH_D = 4                                   # one-hot buffer depth (stages)
PF = 3                                       # one-hot prefetch distance

_CACHE = {}


def _wrap_idx(idx_flat):
    w16 = idx_flat.reshape(-1, 16).T
    return np.ascontiguousarray(np.tile(w16, (8, 1)))


def _build_nc(struct):
    """struct: (caps_b (tuple of 4 tuples), npos_b (tuple), ) shared across
    cores. Returns compiled Bacc."""
    import os as _os
    key = ("nc2", struct, _os.environ.get("KN_GQ"), _os.environ.get("KN_RST"),
           _os.environ.get("KN_PSD"))
    if key in _CACHE:
        return _CACHE[key]
    from contextlib import ExitStack

    from concourse import bacc, mybir, tile

    caps_b = struct
    NB = [len(c) for c in caps_b]
    NPOS = sum(NB)
    reg = [sum(c) for c in caps_b]            # each a multiple of 1024
    TOT = sum(reg)
    COLS = TOT // 128
    NSTAGES = TOT // STAGE

    nc = bacc.Bacc(
        "TRN2",
        target_bir_lowering=False,
        debug=False,
        num_devices=N_CORES,
        num_swdge_queues=4,
    )
    xd_c = [
        nc.dram_tensor(f"xd{b}", [BCHUNK, HALF], mybir.dt.bfloat16,
                       kind="ExternalInput").ap()
        for b in range(N_BETA)
    ]
    xs_d = [
        nc.dram_tensor(f"xs{b}", [128, NB[b] * HALF], mybir.dt.bfloat16,
                       kind="ExternalInput").ap()
        for b in range(N_BETA)
    ]
    dsti_d = nc.dram_tensor("dsti", [128, TOT // 16], mybir.dt.int16,
                            kind="ExternalInput").ap()
    oneh_d = nc.dram_tensor("oneh", [128, TOT], mybir.dt.float8e4,
                            kind="ExternalInput").ap()
    out_d = nc.dram_tensor("out", [128, COLS], mybir.dt.float32,
                           kind="ExternalOutput").ap()

    # ---- shared slot structure -------------------------------------------
    # positions: list of (beta, pos_in_beta, slot_start, cap)
    positions = []
    off = 0
    for b in range(N_BETA):
        for j, cap in enumerate(caps_b[b]):
            positions.append((b, j, off, cap))
            off += cap
    assert off == TOT

    # slabs: per region, chunks of SLAB slots
    slabs = []                                # (beta, slot_start, n)
    off = 0
    for b in range(N_BETA):
        r = reg[b]
        s = 0
        while s < r:
            n = min(SLAB, r - s)
            slabs.append((b, off + s, n))
            s += n
        off += r

    # pieces per stage: stage g -> list of (pos_index, colstart, m, p0, ti)
    # PE tile-position rule: psum base partition p0 must be 0/32/64/96 with
    # max M of 128/32/64/32 respectively (cells are 32-aligned by caps).
    MAXM = {0: 128, 32: 32, 64: 64, 96: 32}
    stage_pieces = [[] for _ in range(NSTAGES)]
    for pi, (b, j, start, cap) in enumerate(positions):
        s = start
        end = start + cap
        while s < end:
            p0 = s % 128
            m = min(MAXM[p0], end - s)
            g = s // STAGE
            stage_pieces[g].append(
                (pi, s - g * STAGE, m, p0, (s // 128) % 8)
            )
            s += m

    # stage -> dslab buffer index
    def slab_of_stage(g):
        s0 = g * STAGE
        for si, (b, st, n) in enumerate(slabs):
            if st <= s0 < st + n:
                return si, (s0 - st) // 128
        raise AssertionError(g)

    pos_base = np.concatenate([[0], np.cumsum(NB)]).astype(int)

    with tile.TileContext(nc) as tc, ExitStack() as ctx:
        fixed = ctx.enter_context(tc.tile_pool(name="fixed", bufs=1))
        psum_pool = ctx.enter_context(
            tc.tile_pool(name="ps", bufs=1, space="PSUM"))

        xs_sb = [fixed.tile([128, NB[b], HALF], mybir.dt.bfloat16,
                            name=f"xs{b}") for b in range(N_BETA)]

        dsti_sb = fixed.tile([128, TOT // 16], mybir.dt.int16)
        dslab = [fixed.tile([128, SLAB // 128, HALF], mybir.dt.bfloat16,
                            name=f"ds{i}") for i in range(8)]
        oneh = [fixed.tile([128, SLAB], mybir.dt.float8e4, name=f"oh{i}")
                for i in range(8)]
        prod = [fixed.tile([128, 8, HALF], mybir.dt.bfloat16, name=f"pr{i}")
                for i in range(2)]
        logits_sb = fixed.tile([128, COLS], mybir.dt.float32)
        PSD = int(_os.environ.get("KN_PSD", "4"))  # psum rotation depth
        psum = [psum_pool.tile([128, 8, HALF], mybir.dt.float32,
                               name=f"pb{i}") for i in range(PSD)]

        # loads: first slab's indices first so gather 0 starts early
        head = SLAB // 16
        nc.sync.dma_start(dsti_sb[:, :head], dsti_d[:, :head])
        nc.sync.dma_start(dsti_sb[:, head:], dsti_d[:, head:])
        for b in range(N_BETA):
            nc.sync.dma_start(xs_sb[b][:], xs_d[b])

        # ---- interleaved gathers + consume stages -------------------------
        # Program order must reflect dataflow: gather(s) is emitted only
        # after the stages consuming dslab[s % 8]'s previous occupant
        # (slab s-8), so the tile framework orders the buffer reuse (WAR)
        # correctly instead of binding early muls to late gathers (RAW on
        # the last writer in program order).
        import os as _os
        GQ = int(_os.environ.get("KN_GQ", "4"))   # queues used
        RST = int(_os.environ.get("KN_RST", "4"))  # reset every RST slabs
        stages_of_slab = [[] for _ in range(len(slabs))]
        for g in range(NSTAGES):
            stages_of_slab[slab_of_stage(g)[0]].append(g)

        def emit_stage(g):
            pt = psum[g % PSD]
            si, toff = slab_of_stage(g)
            lbase = g * STAGE - slabs[si][1]      # stage offset within slab
            for (pi, colstart, m, p0, ti) in stage_pieces[g]:
                b = positions[pi][0]
                pos = pi - pos_base[b]
                nc.tensor.matmul(
                    pt[p0:p0 + m, ti, :],
                    oneh[si % 8][:, lbase + colstart:lbase + colstart + m],
                    xs_sb[b][:, pos, :],
                    start=True, stop=True,
                    tile_position=(0, p0),
                )
            pr = prod[g % 2]
            nc.vector.tensor_mul(
                pr[:].rearrange("p a b -> p (a b)"),
                pt[:].rearrange("p a b -> p (a b)"),
                dslab[si % 8][:, toff:toff + 8, :].rearrange(
                    "p a b -> p (a b)"))
            nc.vector.tensor_reduce(
                logits_sb[:, g * 8:(g + 1) * 8], pr[:],
                axis=mybir.AxisListType.X, op=mybir.AluOpType.add,
            )

        def emit_gather(si):
            b, s0, n = slabs[si]
            nc.sync.dma_start(oneh[si % 8][:, :n], oneh_d[:, s0:s0 + n])
            if si > 0 and si % RST == 0:
                nc.gpsimd.dma_reset()
            nc.gpsimd.dma_gather(
                dslab[si % 8][:, :n // 128, :], xd_c[b],
                dsti_sb[:, s0 // 16:(s0 + n) // 16],
                num_idxs=n, num_idxs_reg=n,
                elem_size=HALF, elem_step=HALF, single_packet=False,
                queue_num=si % GQ,
            )

        for si in range(len(slabs)):
            if si >= 8:
                for g in stages_of_slab[si - 8]:
                    emit_stage(g)
            emit_gather(si)
        for si in range(max(0, len(slabs) - 8), len(slabs)):
            for g in stages_of_slab[si]:
                emit_stage(g)

        nc.scalar.activation(
            logits_sb[:], logits_sb[:], mybir.ActivationFunctionType.Sigmoid)
        nc.sync.dma_start(out_d, logits_sb[:])

    nc.compile()
    _CACHE[key] = nc
    return nc


def _make_run_data(x, edge_label_index):
    import ml_dtypes

    x = np.asarray(x, dtype=np.float32)
    eli = np.asarray(edge_label_index)
    assert x.shape == (N_NODES, 2 * HALF), x.shape
    assert eli.shape == (2, N_EDGES), eli.shape
    src = np.ascontiguousarray(eli[0]).astype(np.int64)
    dst = np.ascontiguousarray(eli[1]).astype(np.int64)
    assert src.min() >= 0 and src.max() < N_NODES
    assert dst.min() >= 0 and dst.max() < N_NODES

    xbf = x.astype(ml_dtypes.bfloat16)
    xd_chunks = [np.ascontiguousarray(xbf[b * BCHUNK:(b + 1) * BCHUNK, HALF:])
                 for b in range(N_BETA)]
    xs_pad = np.zeros((N_TILES * TILE, HALF), ml_dtypes.bfloat16)
    xs_pad[:N_NODES] = xbf[:, :HALF]
    xs_tiles = xs_pad.reshape(N_TILES, TILE, HALF)   # [t, row, feat]

    tile_id = (src >> 7).astype(np.int64)
    beta = dst // BCHUNK
    key = beta * N_TILES + tile_id
    order = np.argsort(key, kind="stable")
    counts = np.bincount(key, minlength=N_BETA * N_TILES)
    cell_start = np.concatenate([[0], np.cumsum(counts)]).astype(np.int64)

    # --- cell assignment: per beta, size-sorted cells dealt to 8 cores ----
    caps_b = []
    grids = []                              # per beta: [NB, 8] tile ids (-1)
    for b in range(N_BETA):
        cnt = counts[b * N_TILES:(b + 1) * N_TILES]
        nz = np.nonzero(cnt)[0]
        srt = nz[np.argsort(-cnt[nz], kind="stable")]
        NBb = (len(srt) + N_CORES - 1) // N_CORES
        grid = -np.ones((NBb, N_CORES), np.int64)
        grid.ravel()[:len(srt)] = srt
        # rank 8j = row max; 32-align so every cell starts on a PE quadrant
        caps = (cnt[grid[:, 0]].astype(np.int64) + 31) // 32 * 32
        # pad region to x1024 by extending the last cap
        tot = int(caps.sum())
        pad = (-tot) % STAGE
        caps[-1] += pad
        caps_b.append(tuple(int(c) for c in caps))
        grids.append(grid)
    struct = tuple(caps_b)

    NB = [len(c) for c in caps_b]
    reg = [sum(c) for c in caps_b]
    TOT = sum(reg)
    COLS = TOT // 128
    reg_base = np.concatenate([[0], np.cumsum(reg)]).astype(np.int64)


    in_maps = []
    edge_core = np.empty(N_EDGES, np.int32)
    edge_slot = np.empty(N_EDGES, np.int64)
    for c in range(N_CORES):
        dsti_flat = np.zeros(TOT, np.int16)
        srcl_flat = np.zeros(TOT, np.int64)
        xs_b = []
        for b in range(N_BETA):
            xs_core = np.zeros((NB[b], TILE, HALF), ml_dtypes.bfloat16)
            off = int(reg_base[b])
            for j in range(NB[b]):
                t = grids[b][j, c]
                cap = caps_b[b][j]
                if t >= 0:
                    xs_core[j] = xs_tiles[t]
                    cs, ce = cell_start[b * N_TILES + t], cell_start[
                        b * N_TILES + t + 1]
                    ed = order[cs:ce]
                    n = len(ed)
                    assert n <= cap, (n, cap)
                    sl = np.arange(off, off + n)
                    edge_core[ed] = c
                    edge_slot[ed] = sl
                    dsti_flat[off:off + n] = (dst[ed] % BCHUNK).astype(
                        np.int16)
                    srcl_flat[off:off + n] = (src[ed] % TILE)
                off += cap
            # [128 rows, NB, 128 feats]
            xs_b.append(np.ascontiguousarray(
                xs_core.transpose(1, 0, 2)).reshape(128, NB[b] * HALF))
        oneh = np.zeros((128, TOT), ml_dtypes.float8_e4m3fn)
        oneh[srcl_flat, np.arange(TOT)] = 1.0
        im = {f"xd{b}": xd_chunks[b] for b in range(N_BETA)}
        im.update({f"xs{b}": xs_b[b] for b in range(N_BETA)})
        im["dsti"] = _wrap_idx(dsti_flat)
        im["oneh"] = oneh
        in_maps.append(im)
    return struct, in_maps, edge_core, edge_slot


def _run(struct, in_maps, **kwargs):
    from concourse.bass_utils import run_bass_kernel_spmd

    nc = _build_nc(struct)
    return run_bass_kernel_spmd(nc, in_maps, core_ids=list(range(N_CORES)),
                                **kwargs)


def kernel(x, edge_label_index):
    struct, in_maps, edge_core, edge_slot = _make_run_data(
        x, edge_label_index)
    res = _run(struct, in_maps)
    parts = [res.results[c]["out"].T.reshape(-1) for c in range(N_CORES)]
    vals = np.stack(parts)                   # [8, TOT]
    return vals[edge_core, edge_slot].reshape(-1, 1).astype(np.float32)
